# revision 1
# baseline (speedup 1.0000x reference)
"""Trainium2 Bass kernel for nn_NNModel_35356170780677.

Spiking RNN: embedding gather -> 2-layer rhythmic-masked recurrence (T=128)
-> vocab decode [4096,512]@[512,32000].

Sharding: recurrence replicated on all 8 cores; decoder vocab dim (32000)
split 8 x 4000. Embedding gather + transposes on host.

Device pipeline (per core), chunked by 8 timesteps:
  - state1 = embT @ fc1 bulk (fp32 PE matmuls, packed-transposed layout)
  - state2 = gates @ (+/-fc2) bulk (float32r split-2 for near-fp32 precision)
  - serial membrane chain on DVE (3 ops/iter fast path), layer2 lags by 16
  - decode: spikes(bf16, exact 0/1) @ dec_w(bf16) trailing the chain
All phases overlap; decode PSUM->SBUF on ACT; spike extraction on GpSimd.
"""

import sys
import types
import numpy as np
import ml_dtypes
from contextlib import ExitStack

import concourse.bass as bass
import concourse.tile as tile
import concourse.bacc as bacc
from concourse import mybir
from concourse.bass_utils import run_bass_kernel_spmd

F32 = mybir.dt.float32
F32R = mybir.dt.float32r
BF16 = mybir.dt.bfloat16
ALU = mybir.AluOpType
AFT = mybir.ActivationFunctionType

T, B, NTOK, NINP, H1, H2 = 128, 32, 32000, 256, 512, 512
NCORES = 8
VSH = NTOK // NCORES            # 4000 vocab per core
TB = T * B                      # 4096
LAG = 16                        # layer2 lags layer1 by LAG iters
ITERS = T + LAG                 # 144
CH = 8                          # iters per chunk
NCHUNK = ITERS // CH            # 18
TH = 0.6
DECAY = 0.6
N_TILES = [(i * 512, min(512, VSH - i * 512)) for i in range((VSH + 511) // 512)]

# module-level knobs / results (used by test harness)
TRACE = False
LAST_EXEC_NS = None
LAST_TRACE_PATH = None
_BUILT = {}


def _install_ntff_hook():
    """Register the NTFF profile hook that the image's antenv lacks."""
    if "antenv.axon_hooks" in sys.modules:
        return
    try:
        import antenv
        mod = types.ModuleType("antenv.axon_hooks")
        mod._hook = None
        mod.set_axon_ntff_profile_hook = lambda h: setattr(mod, "_hook", h)
        mod.get_axon_ntff_profile_hook = lambda: mod._hook
        sys.modules["antenv.axon_hooks"] = mod
        antenv.axon_hooks = mod
        from trn_agent_boot.trn_boot import _ntff_profile_via_ctypes
        mod._hook = _ntff_profile_via_ctypes("/opt/axon/libaxon_pjrt.so")
        import concourse.bass_utils as bu
        bu.upload_artifacts = lambda tmpdir: f"local://{tmpdir}"
    except Exception:
        pass


def build(general: bool):
    nc = bacc.Bacc("TRN2", target_bir_lowering=False, debug=False,
                   enable_asserts=True, num_devices=NCORES)
    embT_d = nc.dram_tensor("embT", [NINP, TB], F32, kind="ExternalInput").ap()
    fc1_d = nc.dram_tensor("fc1", [NINP, H1], F32, kind="ExternalInput").ap()
    fc2_d = nc.dram_tensor("fc2e", [H1, H2], F32, kind="ExternalInput").ap()
    bias_d = nc.dram_tensor("bias", [128, 4], F32, kind="ExternalInput").ap()
    decw_d = nc.dram_tensor("decwT", [H2, VSH], BF16, kind="ExternalInput").ap()
    if general:
        mcat_d = nc.dram_tensor("mcat", [128, ITERS * 256], F32,
                                kind="ExternalInput").ap()
        mbcat_d = nc.dram_tensor("mbcat", [128, ITERS * 256], F32,
                                 kind="ExternalInput").ap()
    out_d = nc.dram_tensor("out", [TB, VSH], F32, kind="ExternalOutput").ap()

    with tile.TileContext(nc, trace_sim=False) as tc:
        with ExitStack() as ctx:
            wp = ctx.enter_context(tc.tile_pool(name="weights", bufs=1))
            tmp = ctx.enter_context(tc.tile_pool(name="tmp", bufs=1))
            embp = ctx.enter_context(
                tc.tile_pool(name="embp", bufs=3 if general else 6))
            scatp = ctx.enter_context(
                tc.tile_pool(name="scatp", bufs=2 if general else 6))
            gp = ctx.enter_context(
                tc.tile_pool(name="gp", bufs=2 if general else 3))
            zp = ctx.enter_context(tc.tile_pool(name="zp", bufs=3))
            memp = ctx.enter_context(tc.tile_pool(name="memp", bufs=3))
            up = ctx.enter_context(tc.tile_pool(name="up", bufs=2))
            obp = ctx.enter_context(tc.tile_pool(name="obp", bufs=6))
            ps1p = ctx.enter_context(tc.tile_pool(name="ps1p", bufs=2, space="PSUM"))
            ps2p = ctx.enter_context(tc.tile_pool(name="ps2p", bufs=2, space="PSUM"))
            pdp = ctx.enter_context(tc.tile_pool(name="pdp", bufs=4, space="PSUM"))
            if general:
                mp = ctx.enter_context(tc.tile_pool(name="mp", bufs=2))

            # ---- resident weights ----
            # DMA order matters: the first state1 matmuls need fc1 + the
            # first emb chunks; bulky decoder weights go last so they don't
            # delay pipeline start (decode begins ~3 chunks in anyway).
            fc1_sb = []
            for kt in range(2):
                t_ = wp.tile([128, H1], F32, tag=f"fc1_{kt}")
                nc.sync.dma_start(t_[:], fc1_d[kt * 128:(kt + 1) * 128, :])
                fc1_sb.append(t_)
            g_init = wp.tile([128, 256], F32, tag="g_init")
            nc.gpsimd.memset(g_init[:], 1.0)
            mem_init = wp.tile([128, 256], F32, tag="mem_init")
            nc.gpsimd.memset(mem_init[:], 0.0)
            # fc1 split into f32r hi+lo (ones path: state1 runs as 3 f32r
            # matmuls per k-tile instead of fp32's 4-cycle/row path)
            fc1_hi, fc1_lo = [], []
            if not general:
                for kt in range(2):
                    hi = wp.tile([128, H1], F32R, tag=f"fc1hi_{kt}")
                    nc.vector.tensor_copy(hi[:], fc1_sb[kt][:])
                    diff = tmp.tile([128, H1], F32, tag="fc1diff")
                    nc.vector.tensor_tensor(diff[:], fc1_sb[kt][:],
                                            hi[:].bitcast(F32), ALU.subtract)
                    lo = wp.tile([128, H1], F32R, tag=f"fc1lo_{kt}")
                    nc.vector.tensor_copy(lo[:], diff[:])
                    fc1_hi.append(hi)
                    fc1_lo.append(lo)

            # ---- per-chunk stores, tracked in python ----
            scat = {}    # state_cat store  [128, 8*256] f32
            g01 = {}     # gate store {0,1} [128, 8*256] f32
            zms = {}     # masked spike cat [128, 8*256] f32 (general)
            g1r = {}     # repacked layer1 spikes [128, 4*256] f32r (j-major)
            z2c = {}     # layer2 spikes, iter-major [128, 8*128] bf16
            z2s = {}     # layer2 spikes, decode layout [128, 1024] bf16
            embt = {}    # emb chunks       2 x [128, 256] f32
            mca = {}     # mask chunks (general)
            mba = {}

            def dma_embt(ec):
                tiles = []
                for kt in range(2):
                    t_ = embp.tile([128, 256], F32, tag=f"embt_{kt}")
                    nc.sync.dma_start(
                        t_[:], embT_d[kt * 128:(kt + 1) * 128,
                                      ec * 256:(ec + 1) * 256])
                    tiles.append(t_)
                embt[ec] = tiles

            def dma_masks(mc):
                mt_ = mp.tile([128, 2048], F32, tag="mcat")
                nc.sync.dma_start(mt_[:], mcat_d[:, mc * 2048:(mc + 1) * 2048])
                mca[mc] = mt_
                bt_ = mp.tile([128, 2048], F32, tag="mbcat")
                nc.sync.dma_start(bt_[:], mbcat_d[:, mc * 2048:(mc + 1) * 2048])
                mba[mc] = bt_

            for ec0 in range(6 if not general else 2):
                dma_embt(ec0)
            if general:
                dma_masks(0)
            # fc2 split into two float32r pieces (hi + lo ~ 26-bit mantissa);
            # needed by state2 matmuls from loop 1 on.
            fc2_hi, fc2_lo = [], []
            for j in range(4):
                raw = tmp.tile([128, H2], F32, tag="fc2raw")
                nc.sync.dma_start(raw[:], fc2_d[j * 128:(j + 1) * 128, :])
                hi = wp.tile([128, H2], F32R, tag=f"fc2hi_{j}")
                nc.vector.tensor_copy(hi[:], raw[:])
                diff = tmp.tile([128, H2], F32, tag="fc2diff")
                nc.vector.tensor_tensor(diff[:], raw[:],
                                        hi[:].bitcast(F32), ALU.subtract)
                lo = wp.tile([128, H2], F32R, tag=f"fc2lo_{j}")
                nc.vector.tensor_copy(lo[:], diff[:])
                fc2_hi.append(hi)
                fc2_lo.append(lo)
            bias_sb = wp.tile([128, 4], F32, tag="bias")
            nc.sync.dma_start(bias_sb[:], bias_d)
            decw_sb = []
            for j in range(4):
                t_ = wp.tile([128, VSH], BF16, tag=f"decw_{j}")
                nc.sync.dma_start(t_[:], decw_d[j * 128:(j + 1) * 128, :])
                decw_sb.append(t_)

            def ensure_scat_l1(ec):
                """Allocate chunk ec's state store + fill its layer1 half."""
                if ec in scat:
                    return
                st = scatp.tile([128, 2048], F32, tag="scat")
                scat[ec] = st
                st5 = st[:].rearrange("p (s h j b) -> p s h j b",
                                      s=8, h=2, j=4, b=32)
                if ec <= 1 or ec >= 16:
                    nc.gpsimd.memset(st[:], 0.0)
                # layer1 states: steps 8*ec + s  (valid ec <= 15)
                if ec <= 15:
                    if general:
                        for j in range(4):
                            ps = ps1p.tile([128, 256], F32, tag="ps1")
                            nc.tensor.matmul(
                                ps[:], fc1_sb[0][:, j * 128:(j + 1) * 128],
                                embt[ec][0][:], start=True, stop=False)
                            nc.tensor.matmul(
                                ps[:], fc1_sb[1][:, j * 128:(j + 1) * 128],
                                embt[ec][1][:], start=False, stop=True)
                            dst = st5[:, :, 0, j, :]
                            src = ps[:].rearrange("p (s b) -> p s b", s=8)
                            nc.scalar.copy(dst, src)
                    else:
                        # split emb chunk into f32r hi+lo; drop lo@lo term
                        # (~2^-26 relative -- far below spike-flip margin)
                        ehi, elo = [], []
                        for kt in range(2):
                            eh = memp.tile([128, 256], F32R, tag=f"eh_{kt}")
                            nc.vector.tensor_copy(eh[:], embt[ec][kt][:])
                            ed = up.tile([128, 256], F32, tag="ed")
                            nc.vector.tensor_tensor(
                                ed[:], embt[ec][kt][:],
                                eh[:].bitcast(F32), ALU.subtract)
                            el = memp.tile([128, 256], F32R, tag=f"el_{kt}")
                            nc.vector.tensor_copy(el[:], ed[:])
                            ehi.append(eh)
                            elo.append(el)
                        for j in range(4):
                            ps = ps1p.tile([128, 256], F32, tag="ps1")
                            terms = [(fc1_hi, ehi), (fc1_lo, ehi),
                                     (fc1_hi, elo)]
                            for ti, (wsp, esp) in enumerate(terms):
                                for kt in range(2):
                                    nc.tensor.matmul(
                                        ps[:],
                                        wsp[kt][:, j * 128:(j + 1) * 128],
                                        esp[kt][:],
                                        start=(ti == 0 and kt == 0),
                                        stop=(ti == 2 and kt == 1))
                            dst = st5[:, :, 0, j, :]
                            src = ps[:].rearrange("p (s b) -> p s b", s=8)
                            nc.scalar.copy(dst, src)

            # pre-build state1 for the first chunks: fills the PE pipe while
            # the serial chain warms up (state1 only needs embeddings)
            if not general:
                for ec0 in range(6):
                    ensure_scat_l1(ec0)

            mem_prev = mem_init
            gate_prev_ap = g_init[:]

            for ic in range(-1, NCHUNK + 1):
                # ---- prefetch DMAs ----
                if 2 <= ic + 2 <= 15 and (ic + 2) not in embt:
                    dma_embt(ic + 2)
                if general and 0 <= ic + 1 <= NCHUNK - 1:
                    dma_masks(ic + 1)

                # ---- build state_cat for chunk ec = ic+1 ----
                ec = ic + 1
                if 0 <= ec <= NCHUNK - 1:
                    ensure_scat_l1(ec)
                    st = scat[ec]
                    st5 = st[:].rearrange("p (s h j b) -> p s h j b",
                                          s=8, h=2, j=4, b=32)
                    # layer2 states: steps 8*ec + s - LAG (valid ec >= 2)
                    if ec >= 2:
                        gc = ec - 2
                        grt = g1r[gc]
                        for ib in range(4):
                            ps = ps2p.tile([128, 256], F32, tag="ps2")
                            for j in range(4):
                                for si, sp in enumerate((fc2_hi, fc2_lo)):
                                    nc.tensor.matmul(
                                        ps[:],
                                        sp[j][:, ib * 128:(ib + 1) * 128],
                                        grt[:, j * 256:(j + 1) * 256],
                                        start=(j == 0 and si == 0),
                                        stop=(j == 3 and si == 1))
                            dst = st5[:, :, 1, ib, :]
                            src = ps[:].rearrange("p (s b) -> p s b", s=8)
                            if general:
                                nc.scalar.copy(dst, src)
                            else:
                                nc.scalar.activation(
                                    dst, src, AFT.Identity,
                                    bias=bias_sb[:, ib:ib + 1], scale=1.0)

                # ---- serial chain for chunk ic ----
                if 0 <= ic <= NCHUNK - 1:
                    gt = gp.tile([128, 2048], F32, tag="g01")
                    g01[ic] = gt
                    if general:
                        zmt = gp.tile([128, 2048], F32, tag="zm")
                        zms[ic] = zmt
                    if ic >= 2:
                        z2ct = zp.tile([128, 1024], BF16, tag="z2c")
                        z2c[ic] = z2ct
                    for s in range(CH):
                        state_ap = scat[ic][:, s * 256:(s + 1) * 256]
                        mem_cur = memp.tile([128, 256], F32, tag="mem")
                        u = up.tile([128, 256], F32, tag="u")
                        nc.vector.scalar_tensor_tensor(
                            u[:], mem_prev[:], DECAY,
                            gate_prev_ap, ALU.mult, ALU.mult)
                        if not general:
                            nc.vector.tensor_tensor(
                                mem_cur[:], u[:], state_ap, ALU.add)
                        else:
                            m_ap = mca[ic][:, s * 256:(s + 1) * 256]
                            mb_ap = mba[ic][:, s * 256:(s + 1) * 256]
                            new = up.tile([128, 256], F32, tag="new")
                            nc.vector.tensor_tensor(
                                new[:], u[:], state_ap, ALU.add)
                            d = up.tile([128, 256], F32, tag="d")
                            nc.vector.tensor_tensor(
                                d[:], new[:], mem_prev[:], ALU.subtract)
                            dm = up.tile([128, 256], F32, tag="dm")
                            nc.vector.tensor_tensor(dm[:], d[:], mb_ap, ALU.mult)
                            nc.vector.tensor_tensor(
                                mem_cur[:], dm[:], mem_prev[:], ALU.add)
                            nc.vector.scalar_tensor_tensor(
                                zmt[:, s * 256:(s + 1) * 256],
                                mem_cur[:], TH, m_ap, ALU.is_gt, ALU.mult)
                            if ic >= 2:
                                nc.scalar.copy(
                                    z2ct[:, s * 128:(s + 1) * 128],
                                    zmt[:, s * 256 + 128:(s + 1) * 256])
                        nc.vector.tensor_scalar(
                            gt[:, s * 256:(s + 1) * 256], mem_cur[:],
                            TH, None, ALU.is_le)
                        if not general and ic >= 2:
                            # spike2 = 1 - gate2, exact affine on ACT
                            nc.scalar.activation(
                                z2ct[:, s * 128:(s + 1) * 128],
                                gt[:, s * 256 + 128:(s + 1) * 256],
                                AFT.Copy, bias=1.0, scale=-1.0)
                        mem_prev = mem_cur
                        gate_prev_ap = gt[:, s * 256:(s + 1) * 256]
                    # repack layer1 gates/spikes j-major + round to f32r
                    if ic <= 15:
                        grt = gp.tile([128, 1024], F32R, tag="g1r")
                        g1r[ic] = grt
                        gsrc = (zms if general else g01)[ic]
                        src = gsrc[:].rearrange(
                            "p (s h j b) -> p s h j b",
                            s=8, h=2, j=4, b=32)[:, :, 0, :, :]
                        dst = grt[:].rearrange(
                            "p (j s b) -> p s j b", j=4, s=8, b=32)
                        nc.vector.tensor_copy(dst, src)
                    # repack layer2 spikes into decode lhsT layout
                    if ic >= 2:
                        z2rt = zp.tile([128, 1024], BF16, tag="z2r")
                        z2s[ic] = z2rt
                        src = z2ct[:].rearrange(
                            "p (mt sl j b) -> p mt j sl b",
                            mt=2, sl=4, j=4, b=32)
                        dst = z2rt[:].rearrange(
                            "p (mt j sl b) -> p mt j sl b",
                            mt=2, j=4, sl=4, b=32)
                        for mt in range(2):
                            nc.scalar.copy(dst[:, mt], src[:, mt])

                # ---- decode chunk (consumes z2s[ic-1]) ----
                if 3 <= ic <= NCHUNK:
                    zt = z2s[ic - 1]
                    row0 = 256 * (ic - 3)
                    for mt in range(2):
                        for (noff, nsz) in N_TILES:
                            ps = pdp.tile([128, 512], F32, tag="psdec")
                            for j in range(4):
                                nc.tensor.matmul(
                                    ps[:, :nsz],
                                    zt[:, mt * 512 + j * 128:
                                       mt * 512 + (j + 1) * 128],
                                    decw_sb[j][:, noff:noff + nsz],
                                    start=(j == 0), stop=(j == 3))
                            ob = obp.tile([128, 512], F32, tag="ob")
                            nc.any.tensor_copy(ob[:, :nsz], ps[:, :nsz])
                            nc.sync.dma_start(
                                out_d[row0 + mt * 128:row0 + (mt + 1) * 128,
                                      noff:noff + nsz],
                                ob[:, :nsz])
    nc.compile()
    return nc


def _get_built(general: bool):
    if general not in _BUILT:
        _BUILT[general] = build(general)
    return _BUILT[general]


def _make_mcat(m1, m2):
    """Iter-indexed replicated mask concat [128, ITERS*256]."""
    out = np.zeros((128, ITERS, 2, 4, 32), np.float32)

    def rep(m):  # [512, T] -> [128, T, 4, 32]
        r = m.reshape(4, 128, T).transpose(1, 2, 0)      # [128, T, 4]
        return np.repeat(r[:, :, :, None], 32, axis=3)

    out[:, :T, 0] = rep(m1)
    out[:, LAG:LAG + T, 1] = rep(m2)
    return np.ascontiguousarray(out.reshape(128, ITERS * 256))


def kernel(**inputs) -> np.ndarray:
    global LAST_EXEC_NS, LAST_TRACE_PATH
    _install_ntff_hook()

    raw = np.asarray(inputs["raw_input"])
    enc_w = np.asarray(inputs["enc_w"], np.float32)
    fc1 = np.asarray(inputs["fc1"], np.float32)
    fc2 = np.asarray(inputs["fc2"], np.float32)
    dec_w = np.asarray(inputs["dec_w"], np.float32)
    dec_b = np.asarray(inputs["dec_b"], np.float32)
    m1 = np.asarray(inputs["mask1"], np.float32)[:, :T]
    m2 = np.asarray(inputs["mask2"], np.float32)[:, :T]

    ones = bool(np.all(m1 == 1.0) and np.all(m2 == 1.0))

    emb = enc_w[raw.reshape(-1).astype(np.int64)]          # [TB, NINP]
    embT = np.ascontiguousarray(emb.T)                     # [NINP, TB]
    fc2_eff = np.ascontiguousarray(-fc2 if ones else fc2)
    if ones:
        colsum = fc2.sum(axis=0, dtype=np.float64).astype(np.float32)  # [H2]
        bias = np.ascontiguousarray(colsum.reshape(4, 128).T)          # [128,4]
    else:
        bias = np.zeros((128, 4), np.float32)
    decwT = np.ascontiguousarray(dec_w.T).astype(ml_dtypes.bfloat16)   # [512,32000]

    in_maps = []
    for c in range(NCORES):
        m = {
            "embT": embT,
            "fc1": np.ascontiguousarray(fc1),
            "fc2e": fc2_eff,
            "bias": bias,
            "decwT": np.ascontiguousarray(decwT[:, c * VSH:(c + 1) * VSH]),
        }
        if not ones:
            if c == 0:
                mcat = _make_mcat(m1, m2)
                mbcat = (mcat != 0).astype(np.float32)
            m["mcat"] = mcat
            m["mbcat"] = mbcat
        in_maps.append(m)

    nc = _get_built(general=not ones)
    res = run_bass_kernel_spmd(nc, in_maps, list(range(NCORES)), trace=TRACE)
    LAST_EXEC_NS = res.exec_time_ns
    if res.instructions_and_trace is not None:
        LAST_TRACE_PATH = res.instructions_and_trace[1]

    out = np.concatenate([res.results[c]["out"] for c in range(NCORES)], axis=1)
    if np.any(dec_b != 0.0):
        out = out + dec_b[None, :]
    return np.ascontiguousarray(out.reshape(T, B, NTOK), dtype=np.float32)

